# revision 16
# baseline (speedup 1.0000x reference)
"""Cross-attention (S2Audio) Trainium2 Bass kernel.

Sharding: data-parallel over the clip batch B=8 -> one batch element per
NeuronCore.  Per core the kernel computes, for its batch element b:

  q = (audio_patch + pos_a) @ q_w.T + q_b          (1568, 768)
  k,v = (s_x_patch + pos_s) @ kv_w.T + kv_b        (1568, 768) each
  out = softmax(q k^T / sqrt(64)) v  per 12 heads  -> proj -> (1568, 768)

Host prep is layout/elementwise only: weight transposes, positional-embedding
combine + add (O(N*D)), bf16 casts, sharding slices.  All matmuls/softmax run
on device.

On-device strategy (PE-array tiling + exp-wall pipelining):
  * matmul operands bf16/fp16; accumulation fp32 in PSUM.
  * activations arrive feature-major (host-transposed) as x_feat [768, tok].
  * K/Q feature-major chunk c holds head 2c on partitions 0-63 and head
    2c+1 on partitions 64-127.
  * scores^T per head have contraction K=64 (head dim) — half the PE.  Row
    tiling (64x128 mode): head 2c at tile_position (0,0), head 2c+1 at
    (64,0) run CONCURRENTLY on the two row-halves of the PE array into two
    separate PSUM banks -> ~2x on scores.  K is zero-padded to 1664 tokens
    so every nk chunk is full-width: every scores matmul keeps the SAME
    tile size (no PE tiling-mode drain), and exp(0)=1 on the 96 pad rows is
    killed by a per-partition bias of -30 fused into the last chunk's exp.
  * exp fused on ScalarE out of PSUM (scale=1/8), fp16, two nk chunks per
    activation (double-wide 2-bank PSUM score tiles).  ScalarE is the
    bottleneck engine (~29.5M exps, partition-parallel only), so the whole
    schedule is built to keep it fed.
  * PV per head is M=64 (head dim out) — again half the PE.  Col tiling
    (128x64 mode): head 2c -> PSUM partitions 0-63 via tile_position
    (0,0), head 2c+1 -> partitions 64-127 via (0,64), accumulated over nk
    chunks in ONE psum tile -> ~2x on PV.
  * two head pairs are processed per tiling-mode phase (all scores of both
    pairs in row mode, then all PVs in col mode) to halve mode switches.
  * softmax denominator: the ones-column trick is incompatible with M=64
    col packing, so: DVE pairwise tree sums the exp tiles -> acc[128, nq]
    per head; one extra col-mode matmul ones[128,64]^T @ acc (same 128x64
    mode, no PE drain) writes the denominator REPLICATED over the head's
    64 PSUM partitions.  One DVE reciprocal + multiply per pair
    normalizes both heads at once.
  * cross-block software pipeline: Q-projection of block b+1 and
    O-projection of block b-1 are emitted inside block b's score windows,
    so ScalarE chews exps while the PE runs full-array projections.
  * O-projection back to token-major fp32, then DMA out.
"""

import numpy as np
from contextlib import ExitStack

B, T, NPATCH, APATCH, D, H = 8, 8, 196, 196, 768, 12
HD = D // H                      # 64
SCALE = float(HD) ** -0.5        # 0.125
NT = NPATCH * T                  # 1568 tokens (same count for q and kv side)
P = 128
DC = D // P                      # 6 feature chunks
N_CORES = 8

# token chunks (partition-dim tiling): 12 x 128 + 1 x 32
TOK_CHUNKS = [(i * P, min(P, NT - i * P)) for i in range((NT + P - 1) // P)]
NTC = len(TOK_CHUNKS)            # 13
NTP = NTC * P                    # 1664: K padded so every chunk is full-width
# nq blocks for the attention/output stage
NQB = 512
NQ_BLOCKS = [(s, min(NQB, NT - s)) for s in range(0, NT, NQB)]

_CACHE: dict = {}


def _build_nc(qb_nz: bool, kb_nz: bool, vb_nz: bool, pb_nz: bool,
              repeat: int = 1):
    import concourse.mybir as mybir
    from concourse import bacc
    from concourse.tile import TileContext

    f32 = mybir.dt.float32
    bf16 = mybir.dt.bfloat16
    fp16 = mybir.dt.float16
    AF = mybir.ActivationFunctionType

    nc = bacc.Bacc("TRN2", target_bir_lowering=False, debug=False,
                   num_devices=N_CORES)

    xsT = nc.dram_tensor("xsT", [D, NT], bf16, kind="ExternalInput")
    xaT = nc.dram_tensor("xaT", [D, NT], bf16, kind="ExternalInput")
    qwT = nc.dram_tensor("qwT", [D, D], bf16, kind="ExternalInput")
    kvwT = nc.dram_tensor("kvwT", [D, 2 * D], bf16, kind="ExternalInput")
    projT = nc.dram_tensor("projT", [D, D], bf16, kind="ExternalInput")
    qb = nc.dram_tensor("qb", [P, DC], f32, kind="ExternalInput") if qb_nz else None
    kb = nc.dram_tensor("kb", [P, DC], f32, kind="ExternalInput") if kb_nz else None
    vb = nc.dram_tensor("vb", [1, D], bf16, kind="ExternalInput") if vb_nz else None
    pb = nc.dram_tensor("pb", [1, D], bf16, kind="ExternalInput") if pb_nz else None
    out = nc.dram_tensor("out", [NT, D], f32, kind="ExternalOutput")

    with TileContext(nc) as tc, ExitStack() as octx:
      for _rep in range(repeat):
        ctx = octx.enter_context(ExitStack())
        consts = ctx.enter_context(tc.tile_pool(name="consts", bufs=1))
        persist = ctx.enter_context(tc.tile_pool(name="persist", bufs=1))

        ones_bf = consts.tile([1, P], bf16, tag="ones_bf")
        nc.gpsimd.memset(ones_bf[:], 1.0)
        # fp16 all-ones stationary for the denominator broadcast matmul
        ones_h = consts.tile([P, HD], fp16, tag="ones_h")
        nc.gpsimd.memset(ones_h[:], 1.0)
        # last token chunk is padded to 128 rows; exp bias -30 on pad rows
        # turns exp(0) for zero-padded K columns into ~0
        ebias = consts.tile([P, 1], f32, tag="ebias")
        nc.vector.memset(ebias[:TOK_CHUNKS[-1][1], :], 0.0)
        nc.vector.memset(ebias[TOK_CHUNKS[-1][1]:2 * TOK_CHUNKS[-1][1], :], -30.0)
        nc.vector.memset(ebias[2 * TOK_CHUNKS[-1][1]:, :], -30.0)
        qb_sb = kb_sb = vb_sb = pb_sb = None
        if qb_nz:
            qb_sb = consts.tile([P, DC], f32, tag="qb")
            nc.sync.dma_start(qb_sb[:], qb[:])
        if kb_nz:
            kb_sb = consts.tile([P, DC], f32, tag="kb")
            nc.sync.dma_start(kb_sb[:], kb[:])
        if vb_nz:
            vb_sb = consts.tile([1, D], bf16, tag="vb")
            nc.sync.dma_start(vb_sb[:], vb[:])
        if pb_nz:
            pb_sb = consts.tile([1, D], bf16, tag="pb")
            nc.sync.dma_start(pb_sb[:], pb[:])

        # persistent SBUF tensors: K (feature-major) and V (token-major)
        k_feat = [persist.tile([P, NTP], bf16, tag=f"k_feat{c}", name=f"k_feat{c}")
                  for c in range(DC)]
        v_st = [persist.tile([P, D], fp16, tag=f"v{i}", name=f"v{i}")
                for i in range(NTC)]

        # ---------------- phase 1: K and V projections ----------------
        with ExitStack() as ph:
            wtp = ph.enter_context(tc.tile_pool(name="wtp", bufs=1))
            xfp = ph.enter_context(tc.tile_pool(name="xfp", bufs=1))
            ps1 = ph.enter_context(tc.tile_pool(name="ps1", bufs=6, space="PSUM"))

            kvw_sb = wtp.tile([P, DC, 2 * D], bf16, tag="kvw", name="kvw")
            nc.sync.dma_start(kvw_sb[:], kvwT.rearrange("(c p) d -> p c d", p=P))

            xs_feat = [xfp.tile([P, NT], bf16, tag=f"xsf{c}", name=f"xsf{c}")
                       for c in range(DC)]
            for c in range(DC):
                nc.sync.dma_start(xs_feat[c][:], xsT[c * P:(c + 1) * P, :])

            # K projection (feature-major)
            for m in range(DC):
                for (n0, nw) in NQ_BLOCKS:
                    ps = ps1.tile([P, NQB], f32, tag="big", name="kproj")
                    for c in range(DC):
                        nc.tensor.matmul(ps[:, :nw],
                                         kvw_sb[:, c, m * P:(m + 1) * P],
                                         xs_feat[c][:, n0:n0 + nw],
                                         start=(c == 0), stop=(c == DC - 1))
                    if kb_nz:
                        nc.scalar.activation(k_feat[m][:, n0:n0 + nw],
                                             ps[:, :nw], AF.Identity,
                                             bias=kb_sb[:, m:m + 1])
                    else:
                        nc.vector.tensor_copy(k_feat[m][:, n0:n0 + nw],
                                              ps[:, :nw])

            for m in range(DC):
                nc.vector.memset(k_feat[m][:, NT:NTP], 0.0)
            nc.vector.memset(v_st[NTC - 1][TOK_CHUNKS[-1][1]:2 * TOK_CHUNKS[-1][1], :], 0.0)
            nc.vector.memset(v_st[NTC - 1][2 * TOK_CHUNKS[-1][1]:, :], 0.0)

            # V projection (token-major, fp16, no ones column)
            for ti, (t0, tw) in enumerate(TOK_CHUNKS):
                for half in range(2):
                    ps = ps1.tile([P, NQB], f32, tag="big", name="vproj")
                    for c in range(DC):
                        nc.tensor.matmul(
                            ps[:tw, :384],
                            xs_feat[c][:, t0:t0 + tw],
                            kvw_sb[:, c, D + half * 384:D + (half + 1) * 384],
                            start=(c == 0), stop=(c == DC - 1 and not vb_nz))
                    if vb_nz:
                        nc.tensor.matmul(
                            ps[:tw, :384], ones_bf[:, :tw],
                            vb_sb[:, half * 384:(half + 1) * 384],
                            start=False, stop=True)
                    nc.vector.tensor_copy(
                        v_st[ti][:tw, half * 384:(half + 1) * 384],
                        ps[:tw, :384])

        # -------- phase 2: per-block Q proj + attention + O-proj --------
        with ExitStack() as ph:
            qwp = ph.enter_context(tc.tile_pool(name="qwp", bufs=1))
            pwp = ph.enter_context(tc.tile_pool(name="pwp", bufs=1))
            xfb = ph.enter_context(tc.tile_pool(name="xfb", bufs=2))
            qfb = ph.enter_context(tc.tile_pool(name="qfb", bufs=2))
            expp = ph.enter_context(tc.tile_pool(name="expp", bufs=28))
            trp2 = ph.enter_context(tc.tile_pool(name="trp2", bufs=8))
            trp = ph.enter_context(tc.tile_pool(name="trp", bufs=6))
            ofp = ph.enter_context(tc.tile_pool(name="ofp", bufs=2))
            otp = ph.enter_context(tc.tile_pool(name="otp", bufs=2))
            nrm = ph.enter_context(tc.tile_pool(name="nrm", bufs=3))
            ps2 = ph.enter_context(tc.tile_pool(name="ps2", bufs=3, space="PSUM"))
            pvps = ph.enter_context(tc.tile_pool(name="pvps", bufs=2, space="PSUM"))

            qw_sb = qwp.tile([P, DC, D], bf16, tag="qw", name="qw")
            nc.sync.dma_start(qw_sb[:], qwT.rearrange("(c p) d -> p c d", p=P))
            pw_sb = pwp.tile([P, DC, D], bf16, tag="pw", name="pw")
            nc.sync.dma_start(pw_sb[:], projT.rearrange("(c p) d -> p c d", p=P))

            def emit_xa_qproj(n0, nw):
                xa_feat = [xfb.tile([P, NQB], bf16, tag=f"xaf{c}", name=f"xaf{c}")
                           for c in range(DC)]
                for c in range(DC):
                    nc.sync.dma_start(xa_feat[c][:, :nw],
                                      xaT[c * P:(c + 1) * P, n0:n0 + nw])
                q_feat = [qfb.tile([P, NQB], bf16, tag=f"qf{c}", name=f"qf{c}")
                          for c in range(DC)]
                for m in range(DC):
                    ps = ps2.tile([P, 2 * NQB], f32, tag="big", name="qproj")
                    for c in range(DC):
                        nc.tensor.matmul(ps[:, :nw],
                                         qw_sb[:, c, m * P:(m + 1) * P],
                                         xa_feat[c][:, :nw],
                                         start=(c == 0), stop=(c == DC - 1))
                    if qb_nz:
                        nc.scalar.activation(q_feat[m][:, :nw], ps[:, :nw],
                                             AF.Identity, bias=qb_sb[:, m:m + 1])
                    else:
                        nc.vector.tensor_copy(q_feat[m][:, :nw], ps[:, :nw])
                return q_feat

            def emit_oproj(n0, nw, out_feat):
                for (c0, cw) in [(c, min(P, nw - c)) for c in range(0, nw, P)]:
                    ot = otp.tile([P, D], f32, tag="ot", name="ot")
                    for half in range(2):
                        ps = ps2.tile([P, 2 * NQB], f32, tag="big", name="oproj")
                        for c in range(DC):
                            nc.tensor.matmul(
                                ps[:cw, :384],
                                out_feat[c][:, c0:c0 + cw],
                                pw_sb[:, c, half * 384:(half + 1) * 384],
                                start=(c == 0), stop=(c == DC - 1 and not pb_nz))
                        if pb_nz:
                            nc.tensor.matmul(
                                ps[:cw, :384], ones_bf[:, :cw],
                                pb_sb[:, half * 384:(half + 1) * 384],
                                start=False, stop=True)
                        nc.vector.tensor_copy(
                            ot[:cw, half * 384:(half + 1) * 384], ps[:cw, :384])
                    nc.sync.dma_start(out[n0 + c0:n0 + c0 + cw, :], ot[:cw, :])

            # software pipeline across nq blocks: Q-proj of block b+1 and
            # O-proj of block b-1 are emitted inside block b's score windows
            # so ScalarE (the exp wall) never starves while the PE runs
            # full-array projection phases.
            q_feat = emit_xa_qproj(*NQ_BLOCKS[0])
            prev = None          # (n0, nw, out_feat) awaiting O-projection
            for bi, (n0, nw) in enumerate(NQ_BLOCKS):
                next_q = None
                out_feat = [ofp.tile([P, NQB], bf16, tag=f"of{c}", name=f"of{c}")
                            for c in range(DC)]
                for pg in range(DC // 2):     # two head pairs per mode phase
                  pair_ids = (2 * pg, 2 * pg + 1)
                  exps = {}
                  # --- scores^T, row-tiled (64x128 mode), both pairs.
                  # Chunks are processed two at a time into double-wide
                  # (2-bank) psum tiles so ONE ScalarE exp covers both.
                  for hp in pair_ids:
                    expA, expB = [], []
                    for j in range((NTC + 1) // 2):
                        c0, c1 = 2 * j, min(2 * j + 1, NTC - 1)
                        two = c1 > c0
                        psA = ps2.tile([P, 2 * NQB], f32, tag="big", name="scA")
                        psB = ps2.tile([P, 2 * NQB], f32, tag="big", name="scB")
                        for ci, cc in enumerate((c0, c1)[:1 + two]):
                            t0 = cc * P
                            nc.tensor.matmul(
                                psA[:, ci * NQB:ci * NQB + nw],
                                k_feat[hp][0:HD, t0:t0 + P],
                                q_feat[hp][0:HD, :nw],
                                start=True, stop=True, tile_position=(0, 0))
                            nc.tensor.matmul(
                                psB[:, ci * NQB:ci * NQB + nw],
                                k_feat[hp][HD:P, t0:t0 + P],
                                q_feat[hp][HD:P, :nw],
                                start=True, stop=True, tile_position=(HD, 0))
                        eA = expp.tile([P, 2 * NQB], fp16, tag="exp", name="expA")
                        eB = expp.tile([P, 2 * NQB], fp16, tag="exp", name="expB")
                        for ps_t, e_t in ((psA, eA), (psB, eB)):
                            if two:
                                src_ap = ps_t[:].rearrange(
                                    "p (c n) -> p c n", c=2)[:, :, :nw]
                                dst_ap = e_t[:].rearrange(
                                    "p (c n) -> p c n", c=2)[:, :, :nw]
                            else:
                                src_ap = ps_t[:, :nw]
                                dst_ap = e_t[:, :nw]
                            if c1 == NTC - 1:
                                # last chunk present: bias kills padded rows
                                nc.scalar.activation(dst_ap, src_ap, AF.Exp,
                                                     scale=SCALE,
                                                     bias=ebias[:, 0:1])
                            else:
                                nc.scalar.activation(dst_ap, src_ap, AF.Exp,
                                                     scale=SCALE)
                        expA.append(eA)
                        expB.append(eB)
                    exps[hp] = (expA, expB)

                  if pg == 0 and prev is not None:
                      emit_oproj(*prev)
                  if pg == DC // 2 - 1 and bi + 1 < len(NQ_BLOCKS):
                      next_q = emit_xa_qproj(*NQ_BLOCKS[bi + 1])

                  # --- PV col-tiled (128x64 mode) + denominators ---
                  for hp in pair_ids:
                    h0, h1 = 2 * hp, 2 * hp + 1
                    expA, expB = exps[hp]
                    pv = pvps.tile([P, NQB], f32, tag="pv", name="pv")
                    for ti in range(NTC):
                        j, ci = ti // 2, ti % 2
                        nc.tensor.matmul(
                            pv[0:HD, :nw],
                            v_st[ti][:, h0 * HD:(h0 + 1) * HD],
                            expA[j][:, ci * NQB:ci * NQB + nw],
                            start=(ti == 0), stop=(ti == NTC - 1),
                            tile_position=(0, 0), skip_group_check=True)
                        nc.tensor.matmul(
                            pv[HD:P, :nw],
                            v_st[ti][:, h1 * HD:(h1 + 1) * HD],
                            expB[j][:, ci * NQB:ci * NQB + nw],
                            start=(ti == 0), stop=(ti == NTC - 1),
                            tile_position=(0, HD), skip_group_check=True)

                    # denominators: DVE tree over 6 double tiles + 1 single
                    accs = []
                    for exp_tiles in (expA, expB):
                        lvl = [t[:].rearrange("p (c n) -> p c n", c=2)[:, :, :nw]
                               for t in exp_tiles[:NTC // 2]]
                        tmp_shape = [P, 2, NQB]
                        while len(lvl) > 1:
                            nxt = []
                            for i in range(0, len(lvl) - 1, 2):
                                s = trp2.tile([P, 2 * NQB], fp16, tag="tr2",
                                             name="tsum")
                                sap = s[:].rearrange("p (c n) -> p c n",
                                                     c=2)[:, :, :nw]
                                nc.vector.tensor_add(sap, lvl[i], lvl[i + 1])
                                nxt.append(sap)
                            if len(lvl) % 2:
                                nxt.append(lvl[-1])
                            lvl = nxt
                        bigacc = lvl[0]
                        acc = trp.tile([P, NQB], fp16, tag="tr", name="acc")
                        nc.vector.tensor_add(acc[:, :nw], bigacc[:, 0, :],
                                             bigacc[:, 1, :])
                        acc2 = trp.tile([P, NQB], fp16, tag="tr", name="acc2")
                        nc.vector.tensor_add(acc2[:, :nw], acc[:, :nw],
                                             exp_tiles[NTC // 2][:, :nw])
                        accs.append(acc2)
                    # denominator broadcast over each head's 64 psum rows via
                    # a col-mode ones matmul (same 128x64 mode as PV)
                    db = pvps.tile([P, NQB], f32, tag="pv", name="db")
                    nc.tensor.matmul(db[0:HD, :nw], ones_h[:, :],
                                     accs[0][:, :nw],
                                     start=True, stop=True,
                                     tile_position=(0, 0),
                                     skip_group_check=True)
                    nc.tensor.matmul(db[HD:P, :nw], ones_h[:, :],
                                     accs[1][:, :nw],
                                     start=True, stop=True,
                                     tile_position=(0, HD),
                                     skip_group_check=True)
                    rb = nrm.tile([P, NQB], f32, tag="rb", name="rb")
                    nc.vector.reciprocal(rb[:, :nw], db[:, :nw])
                    nc.vector.tensor_mul(out_feat[hp][:, :nw],
                                         pv[:, :nw], rb[:, :nw])
                prev = (n0, nw, out_feat)
                if next_q is not None:
                    q_feat = next_q
            emit_oproj(*prev)

        ctx.close()

    nc.finalize()
    return nc


def prep(inputs, repeat: int = 1):
    """Build (cached) nc + per-core input maps from full inputs."""
    import ml_dtypes
    bf = ml_dtypes.bfloat16

    s_x = np.asarray(inputs["s_x"], np.float32)
    audio = np.asarray(inputs["audio"], np.float32)
    q_w = np.asarray(inputs["q_w"], np.float32)
    q_b = np.asarray(inputs["q_b"], np.float32)
    kv_w = np.asarray(inputs["kv_w"], np.float32)
    kv_b = np.asarray(inputs["kv_b"], np.float32)
    proj_w = np.asarray(inputs["proj_w"], np.float32)
    proj_b = np.asarray(inputs["proj_b"], np.float32)

    # host prep: layout + O(N*D) positional add + bf16 casts only
    pos_s = (np.asarray(inputs["clip_space_pos"], np.float32)[:, None, :]
             + np.asarray(inputs["clip_temporal_pos"], np.float32)[None, :, :]
             ).reshape(NT, D)
    pos_a = (np.asarray(inputs["audio_space_pos"], np.float32)[:, None, :]
             + np.asarray(inputs["audio_temporal_pos"], np.float32)[None, :, :]
             ).reshape(NT, D)
    qwT = np.ascontiguousarray(q_w.T).astype(bf)
    kvwT = np.ascontiguousarray(kv_w.T).astype(bf)
    projT = np.ascontiguousarray(proj_w.T).astype(bf)
    qb_nz = bool(np.any(q_b))
    kb_nz = bool(np.any(kv_b[:D]))
    vb_nz = bool(np.any(kv_b[D:]))
    pb_nz = bool(np.any(proj_b))

    key = (qb_nz, kb_nz, vb_nz, pb_nz, repeat)
    if key not in _CACHE:
        _CACHE[key] = _build_nc(qb_nz, kb_nz, vb_nz, pb_nz, repeat=repeat)
    nc = _CACHE[key]

    shared = {"qwT": qwT, "kvwT": kvwT, "projT": projT}
    if qb_nz:
        shared["qb"] = np.ascontiguousarray(q_b.reshape(DC, P).T)
    if kb_nz:
        shared["kb"] = np.ascontiguousarray(kv_b[:D].reshape(DC, P).T)
    if vb_nz:
        shared["vb"] = np.ascontiguousarray(kv_b[D:].reshape(1, D)).astype(bf)
    if pb_nz:
        shared["pb"] = np.ascontiguousarray(proj_b.reshape(1, D)).astype(bf)

    in_maps = []
    for b in range(N_CORES):
        m = dict(shared)
        m["xsT"] = np.ascontiguousarray(
            (s_x[1:, b * T:(b + 1) * T, :].reshape(NT, D) + pos_s).T).astype(bf)
        m["xaT"] = np.ascontiguousarray(
            (audio[2:, b * T:(b + 1) * T, :].reshape(NT, D) + pos_a).T).astype(bf)
        in_maps.append(m)
    return nc, in_maps


def gather(results, inputs):
    """Assemble full output from per-core result dicts."""
    audio = np.asarray(inputs["audio"], np.float32)
    out_full = np.empty((2 + APATCH, B * T, D), np.float32)
    out_full[:2] = audio[:2]
    for b in range(N_CORES):
        out_full[2:, b * T:(b + 1) * T, :] = \
            results[b]["out"].reshape(APATCH, T, D)
    return out_full


def kernel(**inputs) -> np.ndarray:
    nc, in_maps = prep(inputs)
    from concourse.bass_utils import run_bass_kernel_spmd
    res = run_bass_kernel_spmd(nc, in_maps, core_ids=list(range(N_CORES)))
    return gather(res.results, inputs)
